# revision 2
# baseline (speedup 1.0000x reference)
"""GatedGCNConv forward on 8 Trainium2 NeuronCores (Bass/Tile), v2.

Design ("identity scatter", feature-partition layout):
- Host permutes nodes: global degree-sort (desc) + round-robin deal across
  the 8 cores, so every core sees the same per-window chunk schedule K_w
  and padding is ~8% instead of 34%.
- Each 128-node window w owns K_w edge-chunks; the edge at (chunk k,
  partition-slot p) always has dst == node p of the window, so the
  scatter matrix is the identity: no per-chunk one-hot builds, no dst
  tables, no Cx gather.  Padded slots gather a zero row of the x table
  (or an A^-1(-A_b) row when A_b != 0) so they contribute exactly 0.
- Everything runs transposed [feature(128-part) x items(free)]:
  projections keep A/B/C/D/E weights as stationary operands, sigmoid
  bias and the BN affine become per-partition ACT scale/bias, and BN
  statistics fall out of fused tensor_tensor_reduce accumulators.
- Per-chunk x-rows are gathered bf16 via indirect DMA and transposed
  SBUF->SBUF with the DMA xbar (no PE transposes).
- Cross-core traffic: one 1KB AllReduce of BN statistics.
"""

import sys

import numpy as np

sys.path.insert(0, "/opt/trn_rl_repo")

import ml_dtypes  # noqa: E402

BF16 = ml_dtypes.bfloat16

N_NODES = 100000
N_EDGES = 600000
D = 128
ED = 16
P = 128
NCORES = 8
NPC = N_NODES // NCORES  # 12500
W = (NPC + P - 1) // P  # 98
NPAD = W * P  # 12544
BN_EPS = 1e-5
ZROW = N_NODES  # index of the all-zeros row in the gather table
NTAB = ((N_NODES + 1 + 127) // 128) * 128  # gather table rows (100096)
GMAX = 4  # chunks per matmul group (one PSUM bank = 512 f32)

_CACHE = {}
last_results = None


def _build(kws, use_ab, collective=True):
    """kws: tuple of K_w per window (same schedule on every core)."""
    import concourse.bass as bass
    import concourse.tile as tile
    from concourse import mybir, bacc
    from concourse.masks import make_identity

    f32 = mybir.dt.float32
    bf16 = mybir.dt.bfloat16
    i32 = mybir.dt.int32
    Act = mybir.ActivationFunctionType
    Alu = mybir.AluOpType

    C_total = int(sum(kws))

    nc = bacc.Bacc("TRN2", target_bir_lowering=False, debug=False, num_devices=NCORES)

    # ---------------- I/O ----------------
    xtab = nc.dram_tensor("xtab", [NTAB, D], bf16, kind="ExternalInput")
    xlocT = nc.dram_tensor("xlocT", [D, NPAD], bf16, kind="ExternalInput")
    srcw = nc.dram_tensor("srcw", [P, C_total], i32, kind="ExternalInput")
    eatw = nc.dram_tensor("eatw", [ED, C_total * P], bf16, kind="ExternalInput")
    waT = nc.dram_tensor("waT", [D, D], bf16, kind="ExternalInput")
    wbT = nc.dram_tensor("wbT", [D, D], bf16, kind="ExternalInput")
    wcT = nc.dram_tensor("wcT", [D, D], bf16, kind="ExternalInput")
    wdT = nc.dram_tensor("wdT", [D, D], bf16, kind="ExternalInput")
    weT = nc.dram_tensor("weT", [ED, D], bf16, kind="ExternalInput")
    cbe_col = nc.dram_tensor("cbe_col", [D, 1], f32, kind="ExternalInput")
    db_col = nc.dram_tensor("db_col", [D, 1], f32, kind="ExternalInput")
    ab_col = nc.dram_tensor("ab_col", [1, D], f32, kind="ExternalInput")
    gcol = nc.dram_tensor("gcol", [D, 1], f32, kind="ExternalInput")
    bcol = nc.dram_tensor("bcol", [D, 1], f32, kind="ExternalInput")
    outT = nc.dram_tensor("outT", [D, NPAD], f32, kind="ExternalOutput")

    with tile.TileContext(nc) as tc:
        with (
            tc.tile_pool(name="consts", bufs=1) as consts,
            tc.tile_pool(name="persist", bufs=1) as persist,
            tc.tile_pool(name="win", bufs=3) as win,
            tc.tile_pool(name="chunk", bufs=3) as chunk,
            tc.tile_pool(name="psA", bufs=2, space="PSUM") as psA,
            tc.tile_pool(name="psB", bufs=2, space="PSUM") as psB,
            tc.tile_pool(name="psG", bufs=2, space="PSUM") as psG,
            tc.tile_pool(name="dram", bufs=1, space="DRAM") as dpool,
        ):
            # ---------------- constants ----------------
            idb = consts.tile([P, P], bf16)
            make_identity(nc, idb[:])
            wa_t = consts.tile([D, D], bf16)
            nc.sync.dma_start(out=wa_t[:], in_=waT[:])
            wb_t = consts.tile([D, D], bf16)
            nc.sync.dma_start(out=wb_t[:], in_=wbT[:])
            wc_t = consts.tile([D, D], bf16)
            nc.sync.dma_start(out=wc_t[:], in_=wcT[:])
            wd_t = consts.tile([D, D], bf16)
            nc.sync.dma_start(out=wd_t[:], in_=wdT[:])
            we_t = consts.tile([ED, D], bf16)
            nc.sync.dma_start(out=we_t[:], in_=weT[:])
            cbe_t = consts.tile([D, 1], f32)
            nc.sync.dma_start(out=cbe_t[:], in_=cbe_col[:])
            db_t = consts.tile([D, 1], f32)
            nc.sync.dma_start(out=db_t[:], in_=db_col[:])
            if use_ab:
                ab_t = consts.tile([1, D], f32)
                nc.sync.dma_start(out=ab_t[:], in_=ab_col[:])
                ones_row = consts.tile([1, GMAX * P], bf16)
                nc.vector.memset(ones_row[:], 1.0)
                ab_tb = consts.tile([1, D], bf16)
                nc.vector.tensor_copy(out=ab_tb[:], in_=ab_t[:])
            g_t = consts.tile([D, 1], f32)
            nc.sync.dma_start(out=g_t[:], in_=gcol[:])
            b_t = consts.tile([D, 1], f32)
            nc.sync.dma_start(out=b_t[:], in_=bcol[:])
            eps_t = consts.tile([P, 1], f32)
            nc.vector.memset(eps_t[:], BN_EPS)
            zero_col = consts.tile([P, 1], f32)
            nc.vector.memset(zero_col[:], 0.0)

            # ---------------- persistent buffers ----------------
            xlT = persist.tile([D, NPAD], bf16)  # x transposed, local nodes
            nc.sync.dma_start(out=xlT[:], in_=xlocT[:])
            cxT = persist.tile([D, NPAD], bf16)  # Cx
            dsT = persist.tile([D, NPAD], bf16)  # sigmoid(Dx)
            opT = persist.tile([D, NPAD], f32)  # pre-BN output

            # running BN stats [sum, sumsq] — ping-pong accumulators
            stat_acc = persist.tile([D, 2, 2], f32)
            nc.vector.memset(stat_acc[:], 0.0)

            # ---------------- phase 1: Cx / sigmoid(Dx) ----------------
            T1 = 512
            nt1 = (NPAD + T1 - 1) // T1
            for t in range(nt1):
                lo = t * T1
                hi = min(NPAD, lo + T1)
                n = hi - lo
                pc = psA.tile([D, T1], f32, space="PSUM", tag="p1c")
                nc.tensor.matmul(
                    out=pc[:, :n], lhsT=wc_t[:], rhs=xlT[:, lo:hi],
                    start=True, stop=True,
                )
                nc.vector.tensor_copy(out=cxT[:, lo:hi], in_=pc[:, :n])
                pd = psB.tile([D, T1], f32, space="PSUM", tag="p1d")
                nc.tensor.matmul(
                    out=pd[:, :n], lhsT=wd_t[:], rhs=xlT[:, lo:hi],
                    start=True, stop=True,
                )
                nc.scalar.activation(
                    out=dsT[:, lo:hi], in_=pd[:, :n], func=Act.Sigmoid, bias=db_t[:]
                )

            # ---------------- phase 2: edge processing ----------------
            base = 0
            for w, kw in enumerate(kws):
                wlo = w * P
                if kw > 0:
                    sidx = win.tile([P, max(kw, 1)], i32, tag="sidx")
                    nc.sync.dma_start(out=sidx[:], in_=srcw[:, base : base + kw])

                    pagg = psG.tile([P, P], f32, space="PSUM", tag="pagg")

                    k0 = 0
                    while k0 < kw:
                        g = min(GMAX, kw - k0)
                        gb = (base + k0) * P
                        # gather + transpose the g chunks of x rows
                        xg = chunk.tile([P, GMAX, P], bf16, tag="xg")
                        xgT = chunk.tile([P, GMAX * P], bf16, tag="xgT")
                        for k in range(g):
                            nc.gpsimd.indirect_dma_start(
                                out=xg[:, k, :],
                                out_offset=None,
                                in_=xtab[:],
                                in_offset=bass.IndirectOffsetOnAxis(
                                    ap=sidx[:, k0 + k : k0 + k + 1], axis=0
                                ),
                            )
                            nc.sync.dma_start(
                                out=xgT[:, k * P : (k + 1) * P],
                                in_=xg[:, k, :],
                                transpose=True,
                            )
                        eat = chunk.tile([ED, GMAX * P], bf16, tag="eat")
                        nc.sync.dma_start(
                            out=eat[:, : g * P], in_=eatw[:, gb : gb + g * P]
                        )
                        # projections: bank A = Ax^T, bank B = (Bx+Ex+Cx)^T
                        pa = psA.tile([D, GMAX * P], f32, space="PSUM", tag="pa")
                        nc.tensor.matmul(
                            out=pa[:, : g * P], lhsT=wa_t[:], rhs=xgT[:, : g * P],
                            start=True, stop=not use_ab,
                        )
                        if use_ab:
                            nc.tensor.matmul(
                                out=pa[:, : g * P],
                                lhsT=ab_tb[:],
                                rhs=ones_row[:, : g * P],
                                start=False, stop=True,
                                skip_group_check=True,
                            )
                        pb = psB.tile([D, GMAX * P], f32, space="PSUM", tag="pb")
                        nc.tensor.matmul(
                            out=pb[:, : g * P], lhsT=wb_t[:], rhs=xgT[:, : g * P],
                            start=True, stop=False,
                        )
                        nc.tensor.matmul(
                            out=pb[:, : g * P], lhsT=we_t[:], rhs=eat[:, : g * P],
                            start=False, stop=False,
                            skip_group_check=True,
                        )
                        nc.tensor.matmul(
                            out=pb[:, : g * P],
                            lhsT=idb[:],
                            rhs=cxT[:, wlo : wlo + P]
                            .reshape([D, 1, P])
                            .to_broadcast([D, g, P]),
                            start=False, stop=True,
                            skip_group_check=True,
                        )
                        # sigma = sigmoid(Bx+Ex+Cx + cbe)
                        sg = chunk.tile([P, GMAX * P], bf16, tag="sg")
                        nc.scalar.activation(
                            out=sg[:, : g * P], in_=pb[:, : g * P],
                            func=Act.Sigmoid, bias=cbe_t[:],
                        )
                        # msg = Ax * sigma
                        msg = chunk.tile([P, GMAX * P], bf16, tag="msg")
                        nc.vector.tensor_tensor(
                            out=msg[:, : g * P], in0=pa[:, : g * P],
                            in1=sg[:, : g * P], op=Alu.mult,
                        )
                        # identity scatter: agg[:, p] += sum_k msg[:, k, p]
                        for k in range(g):
                            nc.tensor.matmul(
                                out=pagg[:],
                                lhsT=idb[:],
                                rhs=msg[:, k * P : (k + 1) * P],
                                start=(k0 + k == 0),
                                stop=(k0 + k == kw - 1),
                            )
                        k0 += g
                    base += kw

                # ---- window flush: opre = agg*dsig + x; BN stat accum ----
                a = w % 2
                b = 1 - a
                if kw > 0:
                    ag1 = win.tile([P, P], f32, tag="ag1")
                    nc.vector.tensor_tensor(
                        out=ag1[:], in0=pagg[:], in1=dsT[:, wlo : wlo + P],
                        op=Alu.mult,
                    )
                    nc.vector.tensor_tensor_reduce(
                        out=opT[:, wlo : wlo + P],
                        in0=ag1[:],
                        in1=xlT[:, wlo : wlo + P],
                        scale=1.0,
                        scalar=stat_acc[:, 0, a : a + 1],
                        op0=Alu.add,
                        op1=Alu.add,
                        accum_out=stat_acc[:, 0, b : b + 1],
                    )
                else:
                    nc.vector.tensor_tensor_reduce(
                        out=opT[:, wlo : wlo + P],
                        in0=xlT[:, wlo : wlo + P],
                        in1=zero_col[:].to_broadcast([P, P]),
                        scale=1.0,
                        scalar=stat_acc[:, 0, a : a + 1],
                        op0=Alu.add,
                        op1=Alu.add,
                        accum_out=stat_acc[:, 0, b : b + 1],
                    )
                sq = win.tile([P, P], f32, tag="sq")
                nc.vector.tensor_tensor_reduce(
                    out=sq[:],
                    in0=opT[:, wlo : wlo + P],
                    in1=opT[:, wlo : wlo + P],
                    scale=1.0,
                    scalar=stat_acc[:, 1, a : a + 1],
                    op0=Alu.mult,
                    op1=Alu.add,
                    accum_out=stat_acc[:, 1, b : b + 1],
                )

            # ---------------- phase 3: BN AllReduce + normalize ----------------
            fin = (len(kws)) % 2  # slot holding the final accumulators
            stat_s = win.tile([P, 2], f32, tag="stat_s")
            nc.vector.tensor_copy(out=stat_s[:], in_=stat_acc[:, :, fin])
            stat_in = dpool.tile([P, 2], f32)
            stat_out = dpool.tile([P, 2], f32)
            nc.sync.dma_start(out=stat_in[:], in_=stat_s[:])
            if collective:
                nc.gpsimd.collective_compute(
                    "AllReduce",
                    Alu.add,
                    replica_groups=[list(range(NCORES))],
                    ins=[stat_in.opt()],
                    outs=[stat_out.opt()],
                )
            else:
                nc.sync.dma_start(out=stat_out.opt(), in_=stat_in.opt())
            stat2 = win.tile([P, 2], f32, tag="stat2")
            nc.sync.dma_start(out=stat2[:], in_=stat_out[:])

            mean = win.tile([P, 1], f32, tag="mean")
            nc.scalar.mul(out=mean[:], in_=stat2[:, 0:1], mul=1.0 / N_NODES)
            msq = win.tile([P, 1], f32, tag="msq")
            nc.scalar.mul(out=msq[:], in_=stat2[:, 1:2], mul=1.0 / N_NODES)
            mm2 = win.tile([P, 1], f32, tag="mm2")
            nc.vector.tensor_tensor(out=mm2[:], in0=mean[:], in1=mean[:], op=Alu.mult)
            var = win.tile([P, 1], f32, tag="var")
            nc.vector.tensor_tensor(out=var[:], in0=msq[:], in1=mm2[:], op=Alu.subtract)
            sd = win.tile([P, 1], f32, tag="sd")
            nc.scalar.activation(out=sd[:], in_=var[:], func=Act.Sqrt, bias=eps_t[:])
            rstd = win.tile([P, 1], f32, tag="rstd")
            nc.vector.reciprocal(out=rstd[:], in_=sd[:])
            scale = win.tile([P, 1], f32, tag="scale")
            nc.vector.tensor_tensor(out=scale[:], in0=g_t[:], in1=rstd[:], op=Alu.mult)
            msc = win.tile([P, 1], f32, tag="msc")
            nc.vector.tensor_tensor(out=msc[:], in0=mean[:], in1=scale[:], op=Alu.mult)
            shift = win.tile([P, 1], f32, tag="shift")
            nc.vector.tensor_tensor(
                out=shift[:], in0=b_t[:], in1=msc[:], op=Alu.subtract
            )

            # out = relu(scale*opre + shift), streamed out transposed
            T3 = 512
            nt3 = (NPAD + T3 - 1) // T3
            for t in range(nt3):
                lo = t * T3
                hi = min(NPAD, lo + T3)
                ow = win.tile([P, T3], f32, tag="ow")
                nc.scalar.activation(
                    out=ow[:, : hi - lo], in_=opT[:, lo:hi],
                    func=Act.Relu, bias=shift[:], scale=scale[:],
                )
                nc.sync.dma_start(out=outT[:, lo:hi], in_=ow[:, : hi - lo])

    return nc


def _prep_inputs(x, edge_index, edge_attr, A_w, A_b, B_w, B_b, C_w, C_b, D_w, D_b,
                 E_w, E_b, gamma, beta):
    """Host-side sharding/layout. Returns (kws, in_maps, flags, node_order)."""
    x = np.asarray(x, np.float32)
    ei = np.asarray(edge_index)
    ea = np.asarray(edge_attr, np.float32)
    src = np.asarray(ei[0], np.int64)
    dst = np.asarray(ei[1], np.int64)

    deg = np.bincount(dst, minlength=N_NODES)
    order = np.argsort(-deg, kind="stable")  # nodes by degree desc
    # round-robin deal: global rank r -> (core r%8, slot r//8)
    node_core = np.empty(N_NODES, np.int64)
    node_slot = np.empty(N_NODES, np.int64)
    ranks = np.arange(N_NODES, dtype=np.int64)
    node_core[order] = ranks % NCORES
    node_slot[order] = ranks // NCORES
    degs_sorted = deg[order]

    # shared chunk schedule: K_w = max degree among any core's window-w nodes
    kws = tuple(int(degs_sorted[NCORES * P * w]) for w in range(W))
    C_total = int(sum(kws))
    chunk_base = np.zeros(W + 1, np.int64)
    np.cumsum(np.asarray(kws, np.int64), out=chunk_base[1:])

    # per-edge placement
    e_order = np.argsort(dst, kind="stable")
    dst_s = dst[e_order]
    src_s = src[e_order].astype(np.int64)
    ea_s = ea[e_order]
    node_start = np.zeros(N_NODES + 1, np.int64)
    np.cumsum(deg, out=node_start[1:])
    k_e = np.arange(N_EDGES, dtype=np.int64) - node_start[dst_s]
    c_e = node_core[dst_s]
    slot_e = node_slot[dst_s]
    w_e = slot_e >> 7
    p_e = slot_e & 127
    chunk_e = chunk_base[w_e] + k_e
    col_e = chunk_e * P + p_e

    use_ab = bool(np.any(np.asarray(A_b, np.float32) != 0))
    # gather table: x rows + padding row(s).  With A_b != 0 the pad row z
    # solves A z = -A_b so padded slots still contribute exactly 0.
    xtab = np.zeros((NTAB, D), np.float32)
    xtab[:N_NODES] = x
    if use_ab:
        try:
            z = np.linalg.solve(np.asarray(A_w, np.float64),
                                -np.asarray(A_b, np.float64))
        except np.linalg.LinAlgError:
            z = np.linalg.lstsq(np.asarray(A_w, np.float64),
                                -np.asarray(A_b, np.float64), rcond=None)[0]
        xtab[ZROW] = z.astype(np.float32)
    xtab = xtab.astype(BF16)

    srcw = np.full((NCORES, P, C_total), ZROW, np.int32)
    srcw[c_e, p_e, chunk_e] = src_s
    eflat = np.zeros((NCORES, C_total * P, ED), np.float32)
    eflat[c_e, col_e] = ea_s
    eatw = np.ascontiguousarray(eflat.transpose(0, 2, 1)).astype(BF16)

    # per-core transposed x (slot order)
    xloc = np.zeros((NCORES, NPAD, D), np.float32)
    xloc[node_core, node_slot] = x
    xlocT = np.ascontiguousarray(xloc.transpose(0, 2, 1)).astype(BF16)

    waT = np.ascontiguousarray(np.asarray(A_w, np.float32).T).astype(BF16)
    wbT = np.ascontiguousarray(np.asarray(B_w, np.float32).T).astype(BF16)
    wcT = np.ascontiguousarray(np.asarray(C_w, np.float32).T).astype(BF16)
    wdT = np.ascontiguousarray(np.asarray(D_w, np.float32).T).astype(BF16)
    weT = np.ascontiguousarray(np.asarray(E_w, np.float32).T).astype(BF16)
    cbe = (np.asarray(B_b, np.float32) + np.asarray(C_b, np.float32)
           + np.asarray(E_b, np.float32)).reshape(D, 1)
    dbc = np.asarray(D_b, np.float32).reshape(D, 1)
    abr = np.asarray(A_b, np.float32).reshape(1, D)
    gcol = np.asarray(gamma, np.float32).reshape(D, 1)
    bcol = np.asarray(beta, np.float32).reshape(D, 1)

    in_maps = []
    for c in range(NCORES):
        in_maps.append({
            "xtab": xtab,
            "xlocT": xlocT[c],
            "srcw": srcw[c],
            "eatw": eatw[c],
            "waT": waT, "wbT": wbT, "wcT": wcT, "wdT": wdT, "weT": weT,
            "cbe_col": cbe, "db_col": dbc, "ab_col": abr,
            "gcol": gcol, "bcol": bcol,
        })
    return kws, in_maps, (use_ab,), (node_core, node_slot)


def kernel(**inputs) -> np.ndarray:
    global last_results
    from concourse.bass_utils import run_bass_kernel_spmd

    kws, in_maps, flags, (node_core, node_slot) = _prep_inputs(**inputs)
    key = (kws, flags)
    if key not in _CACHE:
        nc = _build(kws, *flags)
        if not nc.is_finalized():
            nc.finalize()
        _CACHE[key] = nc
    nc = _CACHE[key]

    res = run_bass_kernel_spmd(nc, in_maps, core_ids=list(range(NCORES)))
    last_results = res
    out = np.empty((N_NODES, D), np.float32)
    for c in range(NCORES):
        oc = np.asarray(res.results[c]["outT"])  # [D, NPAD]
        mask = node_core == c
        out[mask] = oc.T[node_slot[mask]]
    return out
